# revision 1
# baseline (speedup 1.0000x reference)
"""AdaptiveLabelPropagation Trainium2 kernel (8 NeuronCores, SPMD).

Design
------
Nodes are sharded contiguously across 8 cores (shard size SH = N/8).
Edges (all 3 etypes concatenated, E_tot = 3M) are assigned to the core
owning their *dst* node, so every per-edge gather of cur[dst] / hn[dst]
reads a core-local table with int16-safe indices (dma_gather requires
int16 row indices < 32768).

Within each core, edge positions are laid out as 8 segments by src
range r (src in [SH*r, SH*(r+1))), and inside a segment grouped by the
within-(src,segment) occurrence index k.  All edges inside one (r, k)
group have distinct src, so a single dma_scatter_add call has no
duplicate indices (HW loses duplicate-index updates within one call —
measured).  Calls for consecutive k on the same acc table are ordered
by Tile's WAW dependency, making cross-call read-modify-write safe.

Per layer:
  gather cur_pad[dst_local] (256B rows)  ->  V = w * G (DVE)
  -> k-split dma_scatter_add into acc_r[src - SH*r]  (8 tables)
  -> compact acc[:, :C] -> rs_in [8*SHP, C] -> ReduceScatter -> shard
  -> cur = 0.5 * next/denom + 0.5 * init  -> cur_pad for next layer.

Edge weights w = sigmoid(ew_etype) * cos(hn[src], hn[dst]) are computed
once in phase B from hn (phase A: Linear+LayerNorm+ReLU+L2norm, then
AllGather so src-range slices of hn are readable everywhere).
total_w is obtained by scattering elem_size=17 in layer 0 (col 16 = w).
"""

import sys

if "/opt/trn_rl_repo" not in sys.path:
    sys.path.insert(0, "/opt/trn_rl_repo")

import numpy as np

import concourse.bacc as bacc
import concourse.tile as tile
from concourse import mybir
from concourse.bass_utils import run_bass_kernel_spmd

F32 = mybir.dt.float32
BF16 = mybir.dt.bfloat16
I16 = mybir.dt.int16

N, D, C, E = 100000, 128, 16, 1000000
NUM_LAYERS, ALPHA = 5, 0.5
EPS_COS, EPS_LN = 1e-8, 1e-5
NCORES = 8
SH = N // NCORES          # 12500 real rows per shard
SHP = 12544               # padded shard rows (98 * 128); row SH.. = trash
NT = SHP // 128           # 98
CURW = 64                 # padded cur row width (f32) -> 256B
DUMMY = SH                # trash row for padded positions
CHUNK_B = 4096            # phase-B positions per chunk (2x 512B gathers)
CHUNK_C = 4096            # phase-C positions per chunk (256B gathers)


# ----------------------------------------------------------------- host prep


def _sigmoid(x):
    return 1.0 / (1.0 + np.exp(-np.float64(x)))


def preprocess(inputs):
    """Returns (in_maps, static_cfg). static_cfg = tuple of k-group sizes."""
    src = np.concatenate(
        [inputs["src_connect"], inputs["src_decorate"], inputs["src_next"]]
    ).astype(np.int64)
    dst = np.concatenate(
        [inputs["dst_connect"], inputs["dst_decorate"], inputs["dst_next"]]
    ).astype(np.int64)
    sig = np.concatenate(
        [
            np.full(E, _sigmoid(inputs["ew_connect"][0]), np.float32),
            np.full(E, _sigmoid(inputs["ew_decorate"][0]), np.float32),
            np.full(E, _sigmoid(inputs["ew_next"][0]), np.float32),
        ]
    )

    core = dst // SH
    rng = src // SH
    # occurrence index k of each edge within its (core, rng, src) bucket
    key = (core * NCORES + rng) * N + src
    order = np.argsort(key, kind="stable")
    ks = key[order]
    run_start = np.r_[True, ks[1:] != ks[:-1]]
    run_ids = np.cumsum(run_start) - 1
    first_pos = np.zeros(run_ids[-1] + 1, np.int64)
    first_pos[run_ids[run_start]] = np.nonzero(run_start)[0]
    occ_sorted = np.arange(len(order)) - first_pos[run_ids]
    occ = np.empty(len(order), np.int64)
    occ[order] = occ_sorted

    KMAX = int(occ.max()) + 1
    # counts per (core, rng, k)
    cnt = np.zeros((NCORES, NCORES, KMAX), np.int64)
    np.add.at(cnt, (core, rng, occ), 1)
    # static k-group sizes: max over (core, rng), rounded to 128
    gsz = ((cnt.max(axis=(0, 1)) + 127) // 128 * 128).astype(np.int64)
    seg_used = int(gsz.sum())
    SEGP = (seg_used + CHUNK_B - 1) // CHUNK_B * CHUNK_B
    NPOS = NCORES * SEGP
    g_off = np.concatenate([[0], np.cumsum(gsz)])

    # absolute position of every edge
    within = np.zeros(len(src), np.int64)
    ccnt = np.zeros((NCORES, NCORES, KMAX), np.int64)
    ordered = np.argsort((core * NCORES + rng) * KMAX + occ, kind="stable")
    gk = ((core * NCORES + rng) * KMAX + occ)[ordered]
    rs = np.r_[True, gk[1:] != gk[:-1]]
    rid = np.cumsum(rs) - 1
    fp = np.zeros(rid[-1] + 1, np.int64)
    fp[rid[rs]] = np.nonzero(rs)[0]
    within[ordered] = np.arange(len(ordered)) - fp[rid]
    pos = core * 0 + rng * SEGP + g_off[occ] + within  # core-local position

    idx_dst = np.full((NCORES, NPOS), DUMMY, np.int16)
    idx_src = np.full((NCORES, NPOS), DUMMY, np.int16)
    scale = np.zeros((NCORES, NPOS), np.float32)
    idx_dst[core, pos] = (dst - core * SH).astype(np.int16)
    idx_src[core, pos] = (src - rng * SH).astype(np.int16)
    scale[core, pos] = sig

    def wrap_idx(a):  # [NPOS] -> [128, NPOS//16] (16-wrap replicated 8x)
        w = a.reshape(-1, 16).T
        return np.ascontiguousarray(np.tile(w, (8, 1)))

    def poslay(a):  # [NPOS] -> [128, NPOS//128] position layout
        return np.ascontiguousarray(a.reshape(-1, 128).T)

    feats = np.asarray(inputs["features"], np.float32)
    init = np.asarray(inputs["init_logits"], np.float32)
    W = np.asarray(inputs["W"], np.float32)
    b = np.asarray(inputs["b"], np.float32)
    gam = np.asarray(inputs["ln_gamma"], np.float32)
    bet = np.asarray(inputs["ln_beta"], np.float32)

    in_maps = []
    for c in range(NCORES):
        lo, hi = c * SH, (c + 1) * SH
        featT = np.zeros((D, SHP), np.float32)
        featT[:, :SH] = feats[lo:hi].T
        curpad0 = np.zeros((SHP, CURW), np.float32)
        curpad0[:SH, :C] = init[lo:hi]
        inith = np.zeros((128, NT * C), np.float32)
        ih = 0.5 * init[lo:hi]  # row = p*NT + t
        rows = np.zeros((SHP, C), np.float32)
        rows[:SH] = ih
        inith[:] = rows.reshape(128, NT * C)
        in_maps.append(
            {
                "featT": featT,
                "wt": np.ascontiguousarray(W.T),
                "brow": np.ascontiguousarray(np.tile(b[None, :], (128, 1))),
                "grow": np.ascontiguousarray(np.tile(gam[None, :], (128, 1))),
                "berow": np.ascontiguousarray(np.tile(bet[None, :], (128, 1))),
                "inith": inith,
                "curpad0": curpad0,
                "idx_dst": wrap_idx(idx_dst[c]),
                "idx_src": wrap_idx(idx_src[c]),
                "scale": poslay(scale[c]),
            }
        )
    return in_maps, tuple(int(x) for x in gsz)


# ------------------------------------------------------------------- builder


def build(nc, gsz):
    """Emit the SPMD program. gsz = static k-group sizes (sum -> SEGP)."""
    seg_used = sum(gsz)
    SEGP = (seg_used + CHUNK_B - 1) // CHUNK_B * CHUNK_B
    NPOS = NCORES * SEGP
    NCH_B = NPOS // CHUNK_B
    NCH_C = NPOS // CHUNK_C
    TIL_B = CHUNK_B // 128
    TIL_C = CHUNK_C // 128
    CPS_B = SEGP // CHUNK_B  # phase-B chunks per segment

    g_off = [0]
    for g in gsz:
        g_off.append(g_off[-1] + g)


    # ---- I/O
    featT = nc.dram_tensor("featT", [D, SHP], F32, kind="ExternalInput")
    wt = nc.dram_tensor("wt", [D, D], F32, kind="ExternalInput")
    brow = nc.dram_tensor("brow", [128, D], F32, kind="ExternalInput")
    grow = nc.dram_tensor("grow", [128, D], F32, kind="ExternalInput")
    berow = nc.dram_tensor("berow", [128, D], F32, kind="ExternalInput")
    inith = nc.dram_tensor("inith", [128, NT * C], F32, kind="ExternalInput")
    curpad0 = nc.dram_tensor("curpad0", [SHP, CURW], F32, kind="ExternalInput")
    idx_dst = nc.dram_tensor("idx_dst", [128, NPOS // 16], I16, kind="ExternalInput")
    idx_src = nc.dram_tensor("idx_src", [128, NPOS // 16], I16, kind="ExternalInput")
    scale = nc.dram_tensor("scale", [128, NPOS // 128], F32, kind="ExternalInput")
    out = nc.dram_tensor("out", [SHP, C], F32, kind="ExternalOutput")

    # ---- internal DRAM
    hn_c = nc.dram_tensor("hn_c", [SHP, D], BF16)
    hn_full = nc.dram_tensor("hn_full", [NCORES * SHP, D], BF16)
    cur_pads = [nc.dram_tensor(f"cur_pad{i}", [SHP, CURW], F32) for i in range(2)]
    accs = [nc.dram_tensor(f"acc{r}", [SHP, 2 * CURW], BF16) for r in range(NCORES)]
    rs_in17 = nc.dram_tensor("rs_in17", [NCORES * SHP, C + 1], F32)
    rs_out17 = nc.dram_tensor("rs_out17", [SHP, C + 1], F32)
    rs_in = nc.dram_tensor("rs_in", [NCORES * SHP, C], F32)
    rs_out = nc.dram_tensor("rs_out", [SHP, C], F32)

    rg = [list(range(NCORES))]
    tc = nc._tc  # TileContext attached by caller

    # =================================================== phase A: features
    with tc.tile_pool(name="pa", bufs=2) as pa, tc.tile_pool(
        name="pa1", bufs=1
    ) as pa1, tc.tile_pool(name="pap", bufs=2, space="PSUM") as pap:
        ft = pa1.tile([128, SHP], F32)
        nc.sync.dma_start(out=ft[:], in_=featT[:])
        wts = pa1.tile([128, D], F32)
        nc.sync.dma_start(out=wts[:], in_=wt[:])
        brs = pa1.tile([128, D], F32)
        nc.sync.dma_start(out=brs[:], in_=brow[:])
        grs = pa1.tile([128, D], F32)
        nc.sync.dma_start(out=grs[:], in_=grow[:])
        bes = pa1.tile([128, D], F32)
        nc.sync.dma_start(out=bes[:], in_=berow[:])
        epsl = pa1.tile([128, 1], F32)
        nc.vector.memset(epsl[:], EPS_LN)

        for t in range(NT):
            ps = pap.tile([128, D], F32)
            nc.tensor.matmul(
                out=ps[:],
                lhsT=ft[:, t * 128 : (t + 1) * 128],
                rhs=wts[:],
                start=True,
                stop=True,
            )
            h = pa.tile([128, D], F32)
            nc.vector.tensor_tensor(
                out=h[:], in0=ps[:], in1=brs[:], op=mybir.AluOpType.add
            )
            stats = pa.tile([128, 6], F32)
            nc.vector.bn_stats(out=stats[:], in_=h[:])
            mv = pa.tile([128, 2], F32)
            nc.vector.bn_aggr(out=mv[:], in_=stats[:])
            std = pa.tile([128, 1], F32)
            nc.scalar.activation(
                out=std[:],
                in_=mv[:, 1:2],
                func=mybir.ActivationFunctionType.Sqrt,
                bias=epsl[:],
                scale=1.0,
            )
            rstd = pa.tile([128, 1], F32)
            nc.vector.reciprocal(out=rstd[:], in_=std[:])
            hc = pa.tile([128, D], F32)
            nc.vector.scalar_tensor_tensor(
                out=hc[:],
                in0=h[:],
                scalar=mv[:, 0:1],
                in1=rstd[:].to_broadcast([128, D]),
                op0=mybir.AluOpType.subtract,
                op1=mybir.AluOpType.mult,
            )
            hg = pa.tile([128, D], F32)
            nc.vector.tensor_tensor(
                out=hg[:], in0=hc[:], in1=grs[:], op=mybir.AluOpType.mult
            )
            hb = pa.tile([128, D], F32)
            nc.vector.tensor_tensor(
                out=hb[:], in0=hg[:], in1=bes[:], op=mybir.AluOpType.add
            )
            hr = pa.tile([128, D], F32)
            nc.vector.tensor_scalar(
                out=hr[:],
                in0=hb[:],
                scalar1=0.0,
                scalar2=None,
                op0=mybir.AluOpType.max,
            )
            sq = pa.tile([128, D], F32)
            nc.vector.tensor_tensor(
                out=sq[:], in0=hr[:], in1=hr[:], op=mybir.AluOpType.mult
            )
            ssum = pa.tile([128, 1], F32)
            nc.vector.tensor_reduce(
                out=ssum[:], in_=sq[:], axis=mybir.AxisListType.X,
                op=mybir.AluOpType.add,
            )
            snrm = pa.tile([128, 1], F32)
            nc.scalar.activation(
                out=snrm[:],
                in_=ssum[:],
                func=mybir.ActivationFunctionType.Sqrt,
            )
            scl = pa.tile([128, 1], F32)
            nc.vector.tensor_scalar(
                out=scl[:],
                in0=snrm[:],
                scalar1=EPS_COS,
                scalar2=None,
                op0=mybir.AluOpType.max,
            )
            rcl = pa.tile([128, 1], F32)
            nc.vector.reciprocal(out=rcl[:], in_=scl[:])
            hnf = pa.tile([128, D], BF16)
            nc.vector.tensor_scalar(
                out=hnf[:],
                in0=hr[:],
                scalar1=rcl[:],
                scalar2=None,
                op0=mybir.AluOpType.mult,
            )
            nc.sync.dma_start(
                out=hn_c[t * 128 : (t + 1) * 128, :], in_=hnf[:]
            )

    nc.gpsimd.collective_compute(
        "AllGather",
        mybir.AluOpType.bypass,
        ins=[hn_c[:]],
        outs=[hn_full[:]],
        replica_groups=rg,
    )

    # =================================================== phase B: edge w
    pb1_cm = tc.tile_pool(name="pb1", bufs=1)
    pb1 = pb1_cm.__enter__()
    w_sb = pb1.tile([128, NPOS // 128], F32)
    scale_sb = pb1.tile([128, NPOS // 128], F32)
    nc.sync.dma_start(out=scale_sb[:], in_=scale[:])

    with tc.tile_pool(name="pb", bufs=3) as pb:
        for k in range(NCH_B):
            seg = k // CPS_B
            ccol = k * TIL_B
            icol = k * (CHUNK_B // 16)
            idd = pb.tile([128, CHUNK_B // 16], I16, tag="idd")
            nc.sync.dma_start(
                out=idd[:], in_=idx_dst[:, icol : icol + CHUNK_B // 16]
            )
            ids = pb.tile([128, CHUNK_B // 16], I16, tag="ids")
            nc.sync.dma_start(
                out=ids[:], in_=idx_src[:, icol : icol + CHUNK_B // 16]
            )
            gd = pb.tile([128, TIL_B, D], BF16, tag="gd")
            nc.gpsimd.dma_gather(
                out_ap=gd[:],
                in_ap=hn_c[:],
                idxs_ap=idd[:],
                num_idxs=CHUNK_B,
                num_idxs_reg=CHUNK_B,
                elem_size=D,
                single_packet=False,
                queue_num=0,
            )
            gs = pb.tile([128, TIL_B, D], BF16, tag="gs")
            nc.gpsimd.dma_gather(
                out_ap=gs[:],
                in_ap=hn_full[seg * SHP : (seg + 1) * SHP, :],
                idxs_ap=ids[:],
                num_idxs=CHUNK_B,
                num_idxs_reg=CHUNK_B,
                elem_size=D,
                single_packet=False,
                queue_num=0,
            )
            nc.vector.tensor_tensor(
                out=gd[:], in0=gd[:], in1=gs[:], op=mybir.AluOpType.mult
            )
            sim = pb.tile([128, TIL_B, 1], F32, tag="sim")
            nc.vector.tensor_reduce(
                out=sim[:], in_=gd[:], axis=mybir.AxisListType.X,
                op=mybir.AluOpType.add,
            )
            nc.vector.tensor_tensor(
                out=w_sb[:, ccol : ccol + TIL_B],
                in0=sim[:, :, 0],
                in1=scale_sb[:, ccol : ccol + TIL_B],
                op=mybir.AluOpType.mult,
            )

    # =================================================== phase C: layers
    pc1_cm = tc.tile_pool(name="pc1", bufs=1)
    pc1 = pc1_cm.__enter__()
    ih_sb = pc1.tile([128, NT * C], F32)
    nc.sync.dma_start(out=ih_sb[:], in_=inith[:])
    rdenom = pc1.tile([128, NT], F32)
    zt = pc1.tile([128, SHP // 128 * 2 * CURW // 8], BF16)
    nc.vector.memset(zt[:], 0.0)

    for layer in range(NUM_LAYERS):
        EW = C + 1 if layer == 0 else C
        src_tab = curpad0 if layer == 0 else cur_pads[(layer - 1) % 2]
        racc = rs_in17 if layer == 0 else rs_in
        rout = rs_out17 if layer == 0 else rs_out

        # zero acc tables
        with tc.tile_pool(name=f"pz{layer}", bufs=2) as pz:
            for r in range(NCORES):
                for q in range(8):
                    w8 = SHP // 128 * 2 * CURW // 8
                    nc.sync.dma_start(
                        out=accs[r][:]
                        .rearrange("(a b) c -> a (b c)", a=128)[
                            :, q * w8 : (q + 1) * w8
                        ],
                        in_=zt[:, :],
                    )

        # gather -> V -> k-split scatters, per segment
        with tc.tile_pool(name=f"pg{layer}", bufs=3) as pg, tc.tile_pool(
            name=f"pv{layer}", bufs=2
        ) as pv:
            chunks = [CHUNK_C] * (SEGP // CHUNK_C)
            if SEGP % CHUNK_C:
                chunks.append(SEGP % CHUNK_C)
            for seg in range(NCORES):
                vseg = pv.tile([128, SEGP // 128, EW], BF16, tag="vseg")
                coff = 0
                for csz in chunks:
                    p0 = seg * SEGP + coff  # absolute position
                    ccol = p0 // 128
                    icol = p0 // 16
                    til = csz // 128
                    idd = pg.tile([128, csz // 16], I16, tag=f"idd{csz}")
                    nc.sync.dma_start(
                        out=idd[:], in_=idx_dst[:, icol : icol + csz // 16]
                    )
                    G = pg.tile([128, til, CURW], F32, tag=f"G{csz}")
                    nc.gpsimd.dma_gather(
                        out_ap=G[:],
                        in_ap=src_tab[:],
                        idxs_ap=idd[:],
                        num_idxs=csz,
                        num_idxs_reg=csz,
                        elem_size=CURW,
                        single_packet=False,
                        queue_num=0,
                    )
                    qc = coff // 128
                    nc.vector.tensor_tensor(
                        out=vseg[:, qc : qc + til, 0:C],
                        in0=G[:, :, 0:C],
                        in1=w_sb[:, ccol : ccol + til]
                        .unsqueeze(2)
                        .to_broadcast([128, til, C]),
                        op=mybir.AluOpType.mult,
                    )
                    if EW == C + 1:
                        nc.vector.tensor_copy(
                            out=vseg[:, qc : qc + til, C : C + 1],
                            in_=w_sb[:, ccol : ccol + til].unsqueeze(2),
                        )
                    coff += csz
                # scatter each k-group of this segment
                for ki, g in enumerate(gsz):
                    for s0 in range(0, g, 4096):
                        sg = min(4096, g - s0)
                        c0 = (g_off[ki] + s0) // 128
                        c1 = c0 + sg // 128
                        iof = (seg * SEGP + g_off[ki] + s0) // 16
                        isc = pg.tile([128, sg // 16], I16, tag="isc")
                        nc.sync.dma_start(
                            out=isc[:], in_=idx_src[:, iof : iof + sg // 16]
                        )
                        nc.gpsimd.dma_scatter_add(
                            out_ap=accs[seg][:, 0:EW],
                            in_ap=vseg[:, c0:c1, :],
                            idxs_ap=isc[:],
                            num_idxs=sg,
                            num_idxs_reg=sg,
                            elem_size=EW,
                            elem_step=2 * CURW,
                            single_packet=False,
                            queue_num=0,
                        )

        # compact acc -> rs_in
        with tc.tile_pool(name=f"pk{layer}", bufs=3) as pk:
            for r in range(NCORES):
                for h in range(2):
                    rows = SHP // 2  # 6272 rows
                    tpp = rows // 128  # 49
                    st = pk.tile([128, tpp, 2 * CURW], BF16, tag="st")
                    nc.sync.dma_start(
                        out=st[:],
                        in_=accs[r][h * rows : (h + 1) * rows, :].rearrange(
                            "(p t) c -> p t c", p=128
                        ),
                    )
                    cp = pk.tile([128, tpp, EW], F32, tag="cp")
                    nc.vector.tensor_copy(out=cp[:], in_=st[:, :, 0:EW])
                    nc.sync.dma_start(
                        out=racc[
                            r * SHP + h * rows : r * SHP + (h + 1) * rows, :
                        ].rearrange("(p t) c -> p t c", p=128),
                        in_=cp[:],
                    )

        nc.gpsimd.collective_compute(
            "ReduceScatter",
            mybir.AluOpType.add,
            ins=[racc[:]],
            outs=[rout[:]],
            replica_groups=rg,
        )

        # epilogue: cur = 0.5 * next * rdenom + inith
        with tc.tile_pool(name=f"pe{layer}", bufs=2) as pe:
            nxt = pe.tile([128, NT, EW], F32)
            nc.sync.dma_start(
                out=nxt[:], in_=rout[:].rearrange("(p t) c -> p t c", p=128)
            )
            if layer == 0:
                tw = nxt[:, :, C : C + 1]
                mask = pe.tile([128, NT, 1], F32)
                nc.vector.tensor_scalar(
                    out=mask[:], in0=tw, scalar1=0.0, scalar2=None,
                    op0=mybir.AluOpType.is_gt,
                )
                ones = pe.tile([128, NT, 1], F32)
                nc.vector.memset(ones[:], 1.0)
                md = pe.tile([128, NT, 1], F32)
                nc.vector.scalar_tensor_tensor(
                    out=md[:], in0=mask[:], scalar=-1.0, in1=ones[:],
                    op0=mybir.AluOpType.mult, op1=mybir.AluOpType.add,
                )  # 1 - mask
                tm = pe.tile([128, NT, 1], F32)
                nc.vector.tensor_tensor(
                    out=tm[:], in0=tw, in1=mask[:], op=mybir.AluOpType.mult
                )
                dn = pe.tile([128, NT, 1], F32)
                nc.vector.tensor_tensor(
                    out=dn[:], in0=tm[:], in1=md[:], op=mybir.AluOpType.add
                )
                nc.vector.reciprocal(out=rdenom[:].unsqueeze(2), in_=dn[:])
            tmp = pe.tile([128, NT, C], F32)
            nc.vector.tensor_tensor(
                out=tmp[:],
                in0=nxt[:, :, 0:C],
                in1=rdenom[:].unsqueeze(2).to_broadcast([128, NT, C]),
                op=mybir.AluOpType.mult,
            )
            cur = pe.tile([128, NT, C], F32)
            nc.vector.scalar_tensor_tensor(
                out=cur[:],
                in0=tmp[:],
                scalar=ALPHA,
                in1=ih_sb[:].rearrange("p (t c) -> p t c", c=C),
                op0=mybir.AluOpType.mult,
                op1=mybir.AluOpType.add,
            )
            if layer < NUM_LAYERS - 1:
                dst_tab = cur_pads[layer % 2]
                nc.sync.dma_start(
                    out=dst_tab[:, 0:C].rearrange("(p t) c -> p t c", p=128),
                    in_=cur[:],
                )
                if layer == 0:
                    # zero the pad columns of both cur_pad tables once
                    for tab in cur_pads:
                        zc = pe.tile([128, NT, CURW - C], F32)
                        nc.vector.memset(zc[:], 0.0)
                        nc.sync.dma_start(
                            out=tab[:, C:CURW].rearrange(
                                "(p t) c -> p t c", p=128
                            ),
                            in_=zc[:],
                        )
            else:
                nc.sync.dma_start(
                    out=out[:].rearrange("(p t) c -> p t c", p=128), in_=cur[:]
                )

    pc1_cm.__exit__(None, None, None)
    pb1_cm.__exit__(None, None, None)


def fix_queue_nums(nc):
    """Align each custom Pool-DMA's SWDGE queue with its Tile-assigned
    DMASW sem lane (shadow-sem rule: one queue per sem)."""
    n = 0
    for inst in nc.inst_map.values():
        if type(inst).__name__ in ("InstDMAGatherAnt", "InstDMAScatterAddAnt"):
            p = getattr(inst, "bass_scheduled_proc", None)
            if p is not None and 11 <= p <= 18:
                inst.queue_num = (p - 11) % 4
                n += 1
    return n


# ------------------------------------------------------------------ runtime

_CACHE = {}


def _get_nc(gsz):
    key = tuple(gsz)
    if key not in _CACHE:
        nc = bacc.Bacc(
            "TRN2",
            target_bir_lowering=False,
            debug=False,
            enable_asserts=False,
            num_devices=NCORES,
            num_swdge_queues=4,
        )
        with tile.TileContext(nc) as tc:
            nc._tc = tc
            build(nc, gsz)
        fix_queue_nums(nc)
        nc.compile()
        _CACHE[key] = nc
    return _CACHE[key]


TRACE = False
LAST_RESULT = None


def _install_ntff_hook():
    """Provide antenv.axon_hooks (absent in this image) so that
    run_bass_kernel_spmd(trace=True) can capture NTFF profiles."""
    import types

    if "antenv.axon_hooks" in sys.modules:
        return
    import antenv

    mod = types.ModuleType("antenv.axon_hooks")
    mod._hook = None

    def set_axon_ntff_profile_hook(h):
        mod._hook = h

    def get_axon_ntff_profile_hook():
        return mod._hook

    mod.set_axon_ntff_profile_hook = set_axon_ntff_profile_hook
    mod.get_axon_ntff_profile_hook = get_axon_ntff_profile_hook
    sys.modules["antenv.axon_hooks"] = mod
    antenv.axon_hooks = mod
    try:
        from trn_agent_boot.trn_boot import _ntff_profile_via_ctypes

        h = _ntff_profile_via_ctypes("/opt/axon/libaxon_pjrt.so")
        if h is not None:
            set_axon_ntff_profile_hook(h)
    except Exception as e:  # degrade to no tracing
        print(f"ntff hook install failed: {e}", file=sys.stderr)


def kernel(**inputs):
    global LAST_RESULT
    if TRACE:
        _install_ntff_hook()
    in_maps, gsz = preprocess(inputs)
    nc = _get_nc(gsz)
    res = run_bass_kernel_spmd(
        nc, in_maps, core_ids=list(range(NCORES)), trace=TRACE
    )
    LAST_RESULT = res
    full = np.zeros((N, C), np.float32)
    for c in range(NCORES):
        full[c * SH : (c + 1) * SH] = res.results[c]["out"][:SH]
    return full



# revision 2
# speedup vs baseline: 1.7310x; 1.7310x over previous
"""AdaptiveLabelPropagation Trainium2 kernel v2 (8 NeuronCores, SPMD).

Design (v2: src-sharded, scatter-free)
--------------------------------------
The baseline was bottlenecked by GPSIMD(Q7) SWDGE descriptor generation:
~5.1M per-index descriptors per core (2 phase-B gathers + 5x(gather +
scatter_add)) at ~7.4ns/idx per Q7 pair = ~18ms of Pool-engine time.

v2 halves the per-edge DMA count and removes the scatter entirely:

* Edges are sharded by *src* core.  Each core fully owns its shard's
  next_logits/total_w accumulation (no ReduceScatter); instead the small
  cur table (12544 x 128 bf16 rows, 16 used + 1 ones col) is AllGathered
  each layer (~3.2MB).
* Positions are grouped by (dst segment, src window): segment = dst//12500
  (so gather indices fit int16 against one 12544-row table slice), window
  = 128 consecutive src nodes.  Each 128-position tile stays within one
  src window; within a tile positions are sorted by dst for HBM locality.
* Per tile, a *weighted one-hot* matrix woh[pos, slot] = w_e * (srcslot
  [pos] == slot) is built on the Vector engine (is_equal against an iota
  constant + broadcast multiply, batched over a whole chunk), then the
  Tensor engine computes acc[slot, :] += woh^T @ G[pos, 0:16] directly
  into a per-window PSUM accumulator (98 windows resident in 4 PSUM
  banks).  total_w falls out of a second matmul against the table's ones
  column (layer 0 only).  No dma_scatter_add anywhere.
* The only per-edge DMA left is one dma_gather per layer (+2 in phase B
  for the cosine similarities), spread round-robin over the 4 SWDGE
  queues so all four Q7 core pairs generate descriptors concurrently.
"""

import sys

if "/opt/trn_rl_repo" not in sys.path:
    sys.path.insert(0, "/opt/trn_rl_repo")

import numpy as np

import concourse.bacc as bacc
import concourse.tile as tile
from concourse import mybir
from concourse.bass_utils import run_bass_kernel_spmd

F32 = mybir.dt.float32
BF16 = mybir.dt.bfloat16
I16 = mybir.dt.int16

N, D, C, E = 100000, 128, 16, 1000000
NUM_LAYERS, ALPHA = 5, 0.5
EPS_COS, EPS_LN = 1e-8, 1e-5
NCORES = 8
SH = N // NCORES          # 12500 real rows per shard
SHP = 12544               # padded shard rows (98 * 128)
NW = SHP // 128           # 98 src windows per shard
NSEG = 8                  # dst segments
NT = NW                   # feature tiles in phase A
CHUNK = 4096              # gather chunk (positions)


# ----------------------------------------------------------------- host prep


def _sigmoid(x):
    return 1.0 / (1.0 + np.exp(-np.float64(x)))


def preprocess(inputs):
    """Returns (in_maps, static_cfg). static_cfg = flattened gsz[seg][win]."""
    src = np.concatenate(
        [inputs["src_connect"], inputs["src_decorate"], inputs["src_next"]]
    ).astype(np.int64)
    dst = np.concatenate(
        [inputs["dst_connect"], inputs["dst_decorate"], inputs["dst_next"]]
    ).astype(np.int64)
    sig = np.concatenate(
        [
            np.full(E, _sigmoid(inputs["ew_connect"][0]), np.float32),
            np.full(E, _sigmoid(inputs["ew_decorate"][0]), np.float32),
            np.full(E, _sigmoid(inputs["ew_next"][0]), np.float32),
        ]
    )

    core = src // SH
    s_local = src - core * SH
    seg = dst // SH
    d_idx = (dst - seg * SH).astype(np.int16)
    win = s_local // 128
    slot = (s_local % 128).astype(np.int16)

    cnt = np.zeros((NCORES, NSEG, NW), np.int64)
    np.add.at(cnt, (core, seg, win), 1)
    gsz = ((cnt.max(axis=0) + 127) // 128 * 128).astype(np.int64)  # [8, 98]
    assert (cnt.sum(axis=(0, 1)) > 0).all()
    off = np.zeros((NSEG, NW), np.int64)
    off.ravel()[1:] = np.cumsum(gsz.ravel())[:-1]
    NPOS = int(gsz.sum())

    # rank of each edge within its (core, seg, win) bucket, dst-sorted
    order = np.lexsort((dst, win, seg, core))
    key = ((core * NSEG + seg) * NW + win)[order]
    rs = np.r_[True, key[1:] != key[:-1]]
    rid = np.cumsum(rs) - 1
    fp = np.zeros(rid[-1] + 1, np.int64)
    fp[rid[rs]] = np.nonzero(rs)[0]
    within = np.empty(len(order), np.int64)
    within[order] = np.arange(len(order)) - fp[rid]

    pos = off[seg, win] + within  # core-local position

    idx_dst = np.zeros((NCORES, NPOS), np.int16)
    idx_src = np.zeros((NCORES, NPOS), np.int16)
    slot_a = np.zeros((NCORES, NPOS), np.int16)
    scale = np.zeros((NCORES, NPOS), np.float32)
    idx_dst[core, pos] = d_idx
    idx_src[core, pos] = s_local.astype(np.int16)
    slot_a[core, pos] = slot
    scale[core, pos] = sig

    def wrap_idx(a):  # [NPOS] -> [128, NPOS//16] (16-wrap replicated 8x)
        w = a.reshape(-1, 16).T
        return np.ascontiguousarray(np.tile(w, (8, 1)))

    def poslay(a, dt):  # [NPOS] -> [128, NPOS//128] position layout
        return np.ascontiguousarray(a.reshape(-1, 128).T.astype(dt))

    feats = np.asarray(inputs["features"], np.float32)
    init = np.asarray(inputs["init_logits"], np.float32)
    W = np.asarray(inputs["W"], np.float32)
    b = np.asarray(inputs["b"], np.float32)
    gam = np.asarray(inputs["ln_gamma"], np.float32)
    bet = np.asarray(inputs["ln_beta"], np.float32)

    iota128 = np.tile(np.arange(128, dtype=np.float32)[None, :], (128, 1))

    cur0full = np.zeros((NCORES * SHP, 128), np.float32)
    for c in range(NCORES):
        cur0full[c * SHP : c * SHP + SH, 0:C] = init[c * SH : (c + 1) * SH]
    cur0full[:, C] = 1.0
    cur0full16 = cur0full.astype(np.dtype("bfloat16") if False else np.float32)
    # bf16 conversion via uint16 truncation-with-round
    cur0bf = _to_bf16(cur0full)

    in_maps = []
    for c in range(NCORES):
        lo, hi = c * SH, (c + 1) * SH
        featT = np.zeros((D, SHP), np.float32)
        featT[:, :SH] = feats[lo:hi].T
        ih = np.zeros((128, NW * C), np.float32)
        ihr = np.zeros((SHP, C), np.float32)
        ihr[:SH] = (1.0 - ALPHA) * init[lo:hi]
        # ih[p, w*C + c] = ihr[128*w + p, c]
        ih[:] = ihr.reshape(NW, 128, C).transpose(1, 0, 2).reshape(128, NW * C)
        in_maps.append(
            {
                "featT": featT,
                "wt": np.ascontiguousarray(W.T),
                "brow": np.ascontiguousarray(np.tile(b[None, :], (128, 1))),
                "grow": np.ascontiguousarray(np.tile(gam[None, :], (128, 1))),
                "berow": np.ascontiguousarray(np.tile(bet[None, :], (128, 1))),
                "iota": _to_bf16(iota128),
                "slotrow": _to_bf16(poslay(slot_a[c], np.float32)),
                "scale": poslay(scale[c], np.float32),
                "idx_dst": wrap_idx(idx_dst[c]),
                "idx_src": wrap_idx(idx_src[c]),
                "ih": ih,
                "cur0full": cur0bf,
            }
        )
    return in_maps, tuple(int(x) for x in gsz.ravel())


def _to_bf16(a):
    """Round-to-nearest-even f32 -> bf16, kept as ml_dtypes/np bfloat16."""
    import ml_dtypes

    return np.asarray(a, np.float32).astype(ml_dtypes.bfloat16)


# ------------------------------------------------------------------- builder


def build(nc, gsz_flat):
    gsz = np.asarray(gsz_flat, np.int64).reshape(NSEG, NW)
    off = np.zeros((NSEG, NW), np.int64)
    off.ravel()[1:] = np.cumsum(gsz.ravel())[:-1]
    NPOS = int(gsz.sum())
    NTIL = NPOS // 128

    # global tile t -> window
    tilewin = np.zeros(NTIL, np.int64)
    for s in range(NSEG):
        for w in range(NW):
            t0 = off[s, w] // 128
            tilewin[t0 : t0 + gsz[s, w] // 128] = w
    first = np.zeros(NTIL, bool)
    last = np.zeros(NTIL, bool)
    for w in range(NW):
        ts = np.nonzero(tilewin == w)[0]
        assert len(ts) > 0
        first[ts[0]] = True
        last[ts[-1]] = True

    # per-seg gather chunks (pos_start, n_pos)
    chunks = []
    for s in range(NSEG):
        p0 = int(off[s, 0])
        send = p0 + int(gsz[s].sum())
        p = p0
        while p < send:
            n = min(CHUNK, send - p)
            chunks.append((s, p, n))
            p += n

    # ---- I/O
    featT = nc.dram_tensor("featT", [D, SHP], F32, kind="ExternalInput")
    wt = nc.dram_tensor("wt", [D, D], F32, kind="ExternalInput")
    brow = nc.dram_tensor("brow", [128, D], F32, kind="ExternalInput")
    grow = nc.dram_tensor("grow", [128, D], F32, kind="ExternalInput")
    berow = nc.dram_tensor("berow", [128, D], F32, kind="ExternalInput")
    iota_d = nc.dram_tensor("iota", [128, 128], BF16, kind="ExternalInput")
    slotrow_d = nc.dram_tensor("slotrow", [128, NTIL], BF16, kind="ExternalInput")
    scale_d = nc.dram_tensor("scale", [128, NTIL], F32, kind="ExternalInput")
    idx_dst = nc.dram_tensor("idx_dst", [128, NPOS // 16], I16, kind="ExternalInput")
    idx_src = nc.dram_tensor("idx_src", [128, NPOS // 16], I16, kind="ExternalInput")
    ih_d = nc.dram_tensor("ih", [128, NW * C], F32, kind="ExternalInput")
    cur0full = nc.dram_tensor("cur0full", [NCORES * SHP, 128], BF16, kind="ExternalInput")
    out = nc.dram_tensor("out", [SHP, C], F32, kind="ExternalOutput")

    # ---- internal DRAM
    hn_c = nc.dram_tensor("hn_c", [SHP, D], BF16)
    hn_full = nc.dram_tensor("hn_full", [NCORES * SHP, D], BF16, addr_space="Shared")
    curloc = [nc.dram_tensor(f"curloc{i}", [SHP, 128], BF16) for i in range(2)]
    curfull = [
        nc.dram_tensor(f"curfull{i}", [NCORES * SHP, 128], BF16, addr_space="Shared")
        for i in range(2)
    ]

    rg = [list(range(NCORES))]
    tc = nc._tc

    # =================================================== phase A: features
    with tc.tile_pool(name="pa", bufs=2) as pa, tc.tile_pool(
        name="pa1", bufs=1
    ) as pa1, tc.tile_pool(name="pap", bufs=2, space="PSUM") as pap:
        ft = pa1.tile([128, SHP], F32)
        nc.sync.dma_start(out=ft[:], in_=featT[:])
        wts = pa1.tile([128, D], F32)
        nc.sync.dma_start(out=wts[:], in_=wt[:])
        brs = pa1.tile([128, D], F32)
        nc.sync.dma_start(out=brs[:], in_=brow[:])
        grs = pa1.tile([128, D], F32)
        nc.sync.dma_start(out=grs[:], in_=grow[:])
        bes = pa1.tile([128, D], F32)
        nc.sync.dma_start(out=bes[:], in_=berow[:])
        epsl = pa1.tile([128, 1], F32)
        nc.vector.memset(epsl[:], EPS_LN)

        for t in range(NT):
            ps = pap.tile([128, D], F32)
            nc.tensor.matmul(
                out=ps[:],
                lhsT=ft[:, t * 128 : (t + 1) * 128],
                rhs=wts[:],
                start=True,
                stop=True,
            )
            h = pa.tile([128, D], F32)
            nc.vector.tensor_tensor(
                out=h[:], in0=ps[:], in1=brs[:], op=mybir.AluOpType.add
            )
            stats = pa.tile([128, 6], F32)
            nc.vector.bn_stats(out=stats[:], in_=h[:])
            mv = pa.tile([128, 2], F32)
            nc.vector.bn_aggr(out=mv[:], in_=stats[:])
            std = pa.tile([128, 1], F32)
            nc.scalar.activation(
                out=std[:],
                in_=mv[:, 1:2],
                func=mybir.ActivationFunctionType.Sqrt,
                bias=epsl[:],
                scale=1.0,
            )
            rstd = pa.tile([128, 1], F32)
            nc.vector.reciprocal(out=rstd[:], in_=std[:])
            hc = pa.tile([128, D], F32)
            nc.vector.scalar_tensor_tensor(
                out=hc[:],
                in0=h[:],
                scalar=mv[:, 0:1],
                in1=rstd[:].to_broadcast([128, D]),
                op0=mybir.AluOpType.subtract,
                op1=mybir.AluOpType.mult,
            )
            hg = pa.tile([128, D], F32)
            nc.vector.tensor_tensor(
                out=hg[:], in0=hc[:], in1=grs[:], op=mybir.AluOpType.mult
            )
            hb = pa.tile([128, D], F32)
            nc.vector.tensor_tensor(
                out=hb[:], in0=hg[:], in1=bes[:], op=mybir.AluOpType.add
            )
            hr = pa.tile([128, D], F32)
            nc.vector.tensor_scalar(
                out=hr[:],
                in0=hb[:],
                scalar1=0.0,
                scalar2=None,
                op0=mybir.AluOpType.max,
            )
            sq = pa.tile([128, D], F32)
            nc.vector.tensor_tensor(
                out=sq[:], in0=hr[:], in1=hr[:], op=mybir.AluOpType.mult
            )
            ssum = pa.tile([128, 1], F32)
            nc.vector.tensor_reduce(
                out=ssum[:], in_=sq[:], axis=mybir.AxisListType.X,
                op=mybir.AluOpType.add,
            )
            snrm = pa.tile([128, 1], F32)
            nc.scalar.activation(
                out=snrm[:],
                in_=ssum[:],
                func=mybir.ActivationFunctionType.Sqrt,
            )
            scl = pa.tile([128, 1], F32)
            nc.vector.tensor_scalar(
                out=scl[:],
                in0=snrm[:],
                scalar1=EPS_COS,
                scalar2=None,
                op0=mybir.AluOpType.max,
            )
            rcl = pa.tile([128, 1], F32)
            nc.vector.reciprocal(out=rcl[:], in_=scl[:])
            hnf = pa.tile([128, D], BF16)
            nc.vector.tensor_scalar(
                out=hnf[:],
                in0=hr[:],
                scalar1=rcl[:],
                scalar2=None,
                op0=mybir.AluOpType.mult,
            )
            nc.sync.dma_start(
                out=hn_c[t * 128 : (t + 1) * 128, :], in_=hnf[:]
            )

    nc.gpsimd.collective_compute(
        "AllGather",
        mybir.AluOpType.bypass,
        ins=[hn_c[:]],
        outs=[hn_full[:]],
        replica_groups=rg,
    )

    # ------------------------------------------------- resident SBUF state
    pr_cm = tc.tile_pool(name="pr", bufs=1)
    pr = pr_cm.__enter__()
    w_sb = pr.tile([128, NTIL], BF16)
    slotrow = pr.tile([128, NTIL], BF16)
    nc.sync.dma_start(out=slotrow[:], in_=slotrow_d[:])
    iota = pr.tile([128, 128], BF16)
    nc.sync.dma_start(out=iota[:], in_=iota_d[:])
    ih_sb = pr.tile([128, NW * C], F32)
    nc.sync.dma_start(out=ih_sb[:], in_=ih_d[:])
    rdenom = pr.tile([128, NW], F32)
    curtile = pr.tile([128, NW, C + 1], BF16)
    nc.vector.memset(curtile[:], 1.0)  # col C stays 1.0 (ones column)

    # =================================================== phase B: edge w
    with tc.tile_pool(name="pb", bufs=3) as pb, tc.tile_pool(
        name="pb1", bufs=1
    ) as pb1:
        scale_sb = pb1.tile([128, NTIL], F32)
        nc.sync.dma_start(out=scale_sb[:], in_=scale_d[:])
        for s, p0, np_ in chunks:
            til = np_ // 128
            t0 = p0 // 128
            icol = p0 // 16
            idd = pb.tile([128, CHUNK // 16], I16, tag="idd")
            nc.sync.dma_start(
                out=idd[:, : np_ // 16], in_=idx_dst[:, icol : icol + np_ // 16]
            )
            ids = pb.tile([128, CHUNK // 16], I16, tag="ids")
            nc.sync.dma_start(
                out=ids[:, : np_ // 16], in_=idx_src[:, icol : icol + np_ // 16]
            )
            gd = pb.tile([128, CHUNK // 128, D], BF16, tag="gd")
            nc.gpsimd.dma_gather(
                out_ap=gd[:, :til],
                in_ap=hn_full[s * SHP : (s + 1) * SHP, :],
                idxs_ap=idd[:, : np_ // 16],
                num_idxs=np_,
                num_idxs_reg=np_,
                elem_size=D,
                single_packet=False,
                queue_num=0,
            )
            gs = pb.tile([128, CHUNK // 128, D], BF16, tag="gs")
            nc.gpsimd.dma_gather(
                out_ap=gs[:, :til],
                in_ap=hn_c[:],
                idxs_ap=ids[:, : np_ // 16],
                num_idxs=np_,
                num_idxs_reg=np_,
                elem_size=D,
                single_packet=False,
                queue_num=0,
            )
            nc.vector.tensor_tensor(
                out=gd[:, :til], in0=gd[:, :til], in1=gs[:, :til],
                op=mybir.AluOpType.mult,
            )
            sim = pb.tile([128, CHUNK // 128, 1], F32, tag="sim")
            nc.vector.tensor_reduce(
                out=sim[:, :til], in_=gd[:, :til], axis=mybir.AxisListType.X,
                op=mybir.AluOpType.add,
            )
            nc.vector.tensor_tensor(
                out=w_sb[:, t0 : t0 + til],
                in0=sim[:, :til, 0],
                in1=scale_sb[:, t0 : t0 + til],
                op=mybir.AluOpType.mult,
            )

    # =================================================== phase C: layers
    pcp_cm = tc.tile_pool(name="pcp", bufs=1, space="PSUM")
    pcp = pcp_cm.__enter__()
    acc_ps = pcp.tile([128, NW, C], F32)
    tw_ps = pcp.tile([128, NW], F32)

    for layer in range(NUM_LAYERS):
        src_tab = cur0full if layer == 0 else curfull[(layer - 1) % 2]
        with tc.tile_pool(name=f"pg{layer}", bufs=6) as pg, tc.tile_pool(
            name=f"poh{layer}", bufs=3
        ) as poh:
            for s, p0, np_ in chunks:
                til = np_ // 128
                t0 = p0 // 128
                icol = p0 // 16
                idd = pg.tile([128, CHUNK // 16], I16, tag="idd")
                nc.sync.dma_start(
                    out=idd[:, : np_ // 16],
                    in_=idx_dst[:, icol : icol + np_ // 16],
                )
                G = pg.tile([128, CHUNK // 128, 128], BF16, tag="G")
                nc.gpsimd.dma_gather(
                    out_ap=G[:, :til],
                    in_ap=src_tab[s * SHP : (s + 1) * SHP, :],
                    idxs_ap=idd[:, : np_ // 16],
                    num_idxs=np_,
                    num_idxs_reg=np_,
                    elem_size=128,
                    single_packet=False,
                    queue_num=0,
                )
                oh = poh.tile([128, CHUNK // 128, 128], BF16, tag="oh")
                nc.vector.tensor_tensor(
                    out=oh[:, :til],
                    in0=iota[:].unsqueeze(1).to_broadcast([128, til, 128]),
                    in1=slotrow[:, t0 : t0 + til]
                    .unsqueeze(2)
                    .to_broadcast([128, til, 128]),
                    op=mybir.AluOpType.is_equal,
                )
                nc.vector.tensor_tensor(
                    out=oh[:, :til],
                    in0=oh[:, :til],
                    in1=w_sb[:, t0 : t0 + til]
                    .unsqueeze(2)
                    .to_broadcast([128, til, 128]),
                    op=mybir.AluOpType.mult,
                )
                for ti in range(til):
                    t = t0 + ti
                    w = int(tilewin[t])
                    nc.tensor.matmul(
                        out=acc_ps[:, w, :],
                        lhsT=oh[:, ti, :],
                        rhs=G[:, ti, 0:C],
                        start=bool(first[t]),
                        stop=bool(last[t]),
                        skip_group_check=True,
                    )
                    if layer == 0:
                        nc.tensor.matmul(
                            out=tw_ps[:, w : w + 1],
                            lhsT=oh[:, ti, :],
                            rhs=G[:, ti, C : C + 1],
                            start=bool(first[t]),
                            stop=bool(last[t]),
                            skip_group_check=True,
                        )

        # ------------------------------------------------- layer epilogue
        with tc.tile_pool(name=f"pe{layer}", bufs=1) as pe:
            if layer == 0:
                mask = pe.tile([128, NW], F32)
                nc.vector.tensor_scalar(
                    out=mask[:], in0=tw_ps[:], scalar1=0.0, scalar2=None,
                    op0=mybir.AluOpType.is_gt,
                )
                ones = pe.tile([128, NW], F32)
                nc.vector.memset(ones[:], 1.0)
                md = pe.tile([128, NW], F32)
                nc.vector.scalar_tensor_tensor(
                    out=md[:], in0=mask[:], scalar=-1.0, in1=ones[:],
                    op0=mybir.AluOpType.mult, op1=mybir.AluOpType.add,
                )
                tm = pe.tile([128, NW], F32)
                nc.vector.tensor_tensor(
                    out=tm[:], in0=tw_ps[:], in1=mask[:],
                    op=mybir.AluOpType.mult,
                )
                dn = pe.tile([128, NW], F32)
                nc.vector.tensor_tensor(
                    out=dn[:], in0=tm[:], in1=md[:], op=mybir.AluOpType.add
                )
                rec = pe.tile([128, NW], F32)
                nc.vector.reciprocal(out=rec[:], in_=dn[:])
                nc.vector.tensor_scalar(
                    out=rdenom[:], in0=rec[:], scalar1=ALPHA, scalar2=None,
                    op0=mybir.AluOpType.mult,
                )
            tmp = pe.tile([128, NW, C], F32)
            nc.vector.tensor_tensor(
                out=tmp[:],
                in0=acc_ps[:],
                in1=rdenom[:].unsqueeze(2).to_broadcast([128, NW, C]),
                op=mybir.AluOpType.mult,
            )
            if layer < NUM_LAYERS - 1:
                nc.vector.tensor_tensor(
                    out=curtile[:, :, 0:C],
                    in0=tmp[:],
                    in1=ih_sb[:].rearrange("p (t c) -> p t c", c=C),
                    op=mybir.AluOpType.add,
                )
                nc.sync.dma_start(
                    out=curloc[layer % 2][:, 0 : C + 1].rearrange(
                        "(t p) c -> p t c", p=128
                    ),
                    in_=curtile[:],
                )
                nc.gpsimd.collective_compute(
                    "AllGather",
                    mybir.AluOpType.bypass,
                    ins=[curloc[layer % 2][:]],
                    outs=[curfull[layer % 2][:]],
                    replica_groups=rg,
                )
            else:
                cur = pe.tile([128, NW, C], F32)
                nc.vector.tensor_tensor(
                    out=cur[:],
                    in0=tmp[:],
                    in1=ih_sb[:].rearrange("p (t c) -> p t c", c=C),
                    op=mybir.AluOpType.add,
                )
                nc.sync.dma_start(
                    out=out[:].rearrange("(t p) c -> p t c", p=128), in_=cur[:]
                )

    pcp_cm.__exit__(None, None, None)
    pr_cm.__exit__(None, None, None)


def fix_queue_nums(nc):
    """Align each custom Pool-DMA's SWDGE queue with its Tile-assigned
    DMASW sem lane (shadow-sem rule: one queue per sem)."""
    n = 0
    for inst in nc.inst_map.values():
        if type(inst).__name__ in ("InstDMAGatherAnt", "InstDMAScatterAddAnt"):
            p = getattr(inst, "bass_scheduled_proc", None)
            if p is not None and 11 <= p <= 18:
                inst.queue_num = (p - 11) % 4
                n += 1
    return n


# ------------------------------------------------------------------ runtime

_CACHE = {}


def _get_nc(cfg):
    key = tuple(cfg)
    if key not in _CACHE:
        nc = bacc.Bacc(
            "TRN2",
            target_bir_lowering=False,
            debug=False,
            enable_asserts=False,
            num_devices=NCORES,
            num_swdge_queues=4,
        )
        with tile.TileContext(nc) as tc:
            nc._tc = tc
            build(nc, cfg)
        fix_queue_nums(nc)
        nc.compile()
        _CACHE[key] = nc
    return _CACHE[key]


TRACE = False
LAST_RESULT = None


def _install_ntff_hook():
    """Provide antenv.axon_hooks (absent in this image) so that
    run_bass_kernel_spmd(trace=True) can capture NTFF profiles."""
    import types

    if "antenv.axon_hooks" in sys.modules:
        return
    import antenv

    mod = types.ModuleType("antenv.axon_hooks")
    mod._hook = None

    def set_axon_ntff_profile_hook(h):
        mod._hook = h

    def get_axon_ntff_profile_hook():
        return mod._hook

    mod.set_axon_ntff_profile_hook = set_axon_ntff_profile_hook
    mod.get_axon_ntff_profile_hook = get_axon_ntff_profile_hook
    sys.modules["antenv.axon_hooks"] = mod
    antenv.axon_hooks = mod
    try:
        from trn_agent_boot.trn_boot import _ntff_profile_via_ctypes

        h = _ntff_profile_via_ctypes("/opt/axon/libaxon_pjrt.so")
        if h is not None:
            set_axon_ntff_profile_hook(h)
    except Exception as e:  # degrade to no tracing
        print(f"ntff hook install failed: {e}", file=sys.stderr)


def kernel(**inputs):
    global LAST_RESULT
    if TRACE:
        _install_ntff_hook()
    in_maps, cfg = preprocess(inputs)
    nc = _get_nc(cfg)
    res = run_bass_kernel_spmd(
        nc, in_maps, core_ids=list(range(NCORES)), trace=TRACE
    )
    LAST_RESULT = res
    full = np.zeros((N, C), np.float32)
    for c in range(NCORES):
        full[c * SH : (c + 1) * SH] = res.results[c]["out"][:SH]
    return full
